# revision 14
# baseline (speedup 1.0000x reference)
"""GAT (2-layer graph attention network) Bass kernel for 8 trn2 NeuronCores.

Sharding: core c owns node rows [512c, 512c+512). Each core projects only its
own 512 nodes (h = x_own @ W1) and all-gathers the augmented per-head blocks;
attention exponentials are computed per j-chunk in transposed layout
[j(partitions), i(free)].

Per-chunk paths (chosen by jc % 4, so the all-gather payload is one block per
chunk) share one PSUM accumulator per head:
  jc%4<3 (ACT path): P = patchedExp(s_src[i] + s_dst[j]) * mask
  jc%4=3 (MAX path): P = max(E2s[i]*qd[j], E1s[i])*E1d[j] * mask
               via exp(lrelu(t)) = max(exp(t), exp(0.2 t)), with exp(s_dst)
               folded into the gathered stationary block (htil).
Layer 2 projects h2 for the local node shard only (from local z1 columns) and
all-gathers the small augmented [h2 | 1 | s2_dst] blocks. The patched ACT exp
table computes exp(lrelu(x)); tables needing a true exp (E1s/E2s/qd/E1d) are
host-side folds of the rank-1 score projections x @ (W1 a1_*). Softmax
reciprocals run as ln/exp on the scalar engine (rec = patchedExp(-5*ln(K*den))
= 1/(K*den)); Exp is pinned to the natural_log_exp_and_others table set so
Exp/Ln share one table load. Both layer-1 phases are emitted before their
epilogues so no in-order engine stream stalls on a PSUM-gated epilogue op
between phases.
"""

import os

import numpy as np

N, FIN, HID, H, D1, C = 4096, 512, 256, 4, 64, 64
NCORES = 8
SH = N // NCORES          # 512 local nodes per core
NB = N // 128             # 32 j-chunks
FC = FIN // 128           # 4 fin chunks
OWN = 4                   # own j-blocks per core
NEG = 0.2
AUGH = D1 + 1             # 65 per head
AUG = AUGH * H            # 260
KREC = 32.0               # reciprocal pre-scale (keeps ln(K*den) in (0, 17))

_CACHED = {}


def _make_act_root(alpha=NEG):
    """Patch the neuron ACT tables so Exp computes g(x)=exp(lrelu(x)).

    Bucket entries are [d0,d1,d2,d3,x0,0,0,0] fp32 cubics evaluated as
    y = d0+(x-x0)(d1+(x-x0)(d2+(x-x0)d3)). For exp buckets centered at
    x0<0 we substitute the Taylor cubic of exp(alpha*x) at the same
    center. Ln buckets are untouched.
    """
    import json
    import shutil
    import tempfile

    from neuronxcc.driver.Job import Job
    from neuronxcc.driver.jobs.support.FindActInfo import findActInfoFile

    src_dir = os.path.dirname(findActInfoFile(Job.getPackageDir(), "gen3"))
    dst = tempfile.mkdtemp(prefix="gat_act_root_")
    for f in os.listdir(src_dir):
        shutil.copy(os.path.join(src_dir, f), os.path.join(dst, f))
        os.chmod(os.path.join(dst, f), 0o644)
    for set_name in ("exp_and_others", "natural_log_exp_and_others",
                     "exp_and_friends"):
        meta = json.load(open(os.path.join(dst, f"{set_name}.json")))
        start = meta["func_to_bkt_start_idx"].get("exp")
        if start is None:
            continue
        nxt = [s for s in sorted(meta["func_to_bkt_start_idx"].values())
               if s > start]
        end = nxt[0] if nxt else meta["bkt_entry_cnt"]
        path = os.path.join(dst, f"{set_name}_bkt.bin")
        b = np.fromfile(path, dtype=np.float32).reshape(-1, 8).copy()
        for i in range(start, end):
            x0, d0 = float(b[i, 4]), float(b[i, 0])
            if x0 >= 0 or not np.isfinite(d0) or d0 <= 0:
                continue
            e = np.exp(alpha * x0)
            b[i, 0:4] = [e, alpha * e, alpha * alpha * e / 2.0,
                         alpha ** 3 * e / 6.0]
        b.tofile(path)
    return os.path.join(dst, "act_info.json")


def _pin_exp_table(act_root, mybir, bacc):
    """Make bacc's table-load pass see Exp only in the set that also holds
    Ln, so the whole kernel uses one ACT table load (no Exp<->Ln flips)."""
    import json

    with open(act_root) as f:
        info = json.load(f)
    tables = {}
    for ent in info["act_func_sets"]:
        fns = set()
        for v in ent["act"].keys():
            try:
                fns.add(mybir.ActivationFunctionType.from_pwp(v))
            except Exception:
                pass
        if ent["name"] != "natural_log_exp_and_others":
            fns.discard(mybir.ActivationFunctionType.Exp)
        tables[ent["name"]] = fns
    bacc.get_activation_tables = lambda arch: tables


def _build_nc():
    act_root = _make_act_root()
    os.environ["BASS_ACT_ROOT_JSON_PATH"] = act_root
    import concourse.mybir as mybir
    import concourse.tile as tile
    from concourse import bacc

    _pin_exp_table(act_root, mybir, bacc)

    f32 = mybir.dt.float32
    f32r = mybir.dt.float32r
    bf16 = mybir.dt.bfloat16
    Af = mybir.ActivationFunctionType
    Alu = mybir.AluOpType

    nc = bacc.Bacc("TRN2", target_bir_lowering=False, debug=False,
                   num_devices=NCORES)

    xsT_d = nc.dram_tensor("xsT", [FIN, SH], f32r, kind="ExternalInput").ap()
    mT_d = nc.dram_tensor("maskT", [N, SH], bf16, kind="ExternalInput").ap()
    W1_d = nc.dram_tensor("W1a", [FIN, HID], f32r, kind="ExternalInput").ap()
    ssrcb_d = nc.dram_tensor("ssrcb", [128, H, SH], bf16, kind="ExternalInput").ap()
    E1sb_d = nc.dram_tensor("E1sb", [128, H, SH], bf16, kind="ExternalInput").ap()
    E2sb_d = nc.dram_tensor("E2sb", [128, H, SH], bf16, kind="ExternalInput").ap()
    sdst_d = nc.dram_tensor("sdstT", [128, NB, H], f32, kind="ExternalInput").ap()
    qd_d = nc.dram_tensor("qdT", [128, NB, H], f32, kind="ExternalInput").ap()
    E1dl_d = nc.dram_tensor("E1dloc", [128, 1, H], f32, kind="ExternalInput").ap()
    W2e_d = nc.dram_tensor("W2e", [HID, C + 1], bf16, kind="ExternalInput").ap()
    v2s_d = nc.dram_tensor("v2s", [HID, 1], bf16, kind="ExternalInput").ap()
    outT_d = nc.dram_tensor("outT", [C, SH], f32, kind="ExternalOutput").ap()

    with tile.TileContext(nc) as tc:
        with (tc.tile_pool(name="persist", bufs=1) as pp,
              tc.tile_pool(name="dram", bufs=1, space="DRAM") as dpool):
            # ---------------- persistent SBUF tiles -----------------------
            maskr = pp.tile([128, NB, SH], bf16)
            h1all = pp.tile([128, NB, AUG], bf16)
            ssrcb = pp.tile([128, H, SH], bf16)
            E1sb = pp.tile([128, H, SH], bf16)
            E2sb = pp.tile([128, H, SH], bf16)
            sdstT = pp.tile([128, NB, H], f32)
            qdT = pp.tile([128, NB, H], f32)
            E1dloc = pp.tile([128, 1, H], f32)
            xsTt = pp.tile([128, FC, SH], f32r)
            W1sb = pp.tile([128, FC, HID], f32r)
            h1loc = pp.tile([128, OWN, AUG], bf16)
            htloc = pp.tile([128, 1, AUG], bf16)
            z1Tl = pp.tile([128, 2, SH], bf16)
            h2l = pp.tile([128, OWN, C + 1], bf16)      # [h2 | 1]
            h2all = pp.tile([128, NB, C + 1], bf16)
            s2dst = pp.tile([128, NB, 1], f32)
            s2dcol = pp.tile([128, OWN], f32)
            pexf = pp.tile([128, NB, SH], bf16)
            s2srow = pp.tile([1, SH], f32)
            s2srcb = pp.tile([128, SH], f32)
            W2sb = pp.tile([128, 2, C + 1], bf16)
            v2sb = pp.tile([128, 2, 1], bf16)
            ones_col = pp.tile([128, 1], bf16)
            rec2row = pp.tile([1, SH], f32)

            ag1_in = dpool.tile([OWN * 128, AUG], bf16)
            ag1_out = dpool.tile([N, AUG], bf16, addr_space="Shared")
            ag3_in = dpool.tile([OWN * 128, C + 1], bf16)
            ag3_out = dpool.tile([N, C + 1], bf16, addr_space="Shared")
            ag4_in = dpool.tile([128, OWN], f32)
            ag4_out = dpool.tile([128 * NCORES, OWN], f32, addr_space="Shared")

            # ---------------- AG1-critical input DMAs ---------------------
            for fc in range(FC):
                nc.sync.dma_start(xsTt[:, fc, :], xsT_d[fc * 128:(fc + 1) * 128, :])
                nc.sync.dma_start(W1sb[:, fc, :], W1_d[fc * 128:(fc + 1) * 128, :])
            nc.sync.dma_start(E1dloc[:], E1dl_d)
            nc.vector.memset(ones_col[:], 1.0)

            # ---------------- local prep: h1aug (+ htil for k=3) ----------
            with tc.tile_pool(name="ppsum", bufs=2, space="PSUM") as ppsum:
                h1v = h1loc[:].rearrange("p k (h x) -> p k h x", x=AUGH)
                nc.vector.tensor_copy(
                    h1v[:, :, :, D1:D1 + 1],
                    ones_col[:].unsqueeze(1).unsqueeze(1).to_broadcast(
                        (128, OWN, H, 1)))
                for k in range(OWN):
                    hp = ppsum.tile([128, HID], f32, tag="hp")
                    for fc in range(FC):
                        nc.tensor.matmul(
                            hp[:], xsTt[:, fc, k * 128:(k + 1) * 128],
                            W1sb[:, fc, :],
                            start=(fc == 0), stop=(fc == FC - 1))
                    nc.vector.tensor_copy(
                        h1v[:, k, :, 0:D1],
                        hp[:].rearrange("p (h d) -> p h d", h=H))
                    if k == OWN - 1:
                        for h in range(H):
                            nc.vector.tensor_scalar_mul(
                                htloc[:, 0, h * AUGH:(h + 1) * AUGH],
                                h1loc[:, k, h * AUGH:(h + 1) * AUGH],
                                E1dloc[:, 0, h:h + 1])
                        nc.sync.dma_start(ag1_in[k * 128:(k + 1) * 128, :],
                                          htloc[:, 0, :])
                    else:
                        nc.sync.dma_start(ag1_in[k * 128:(k + 1) * 128, :],
                                          h1loc[:, k, :])

            nc.gpsimd.collective_compute(
                "AllGather", Alu.bypass,
                replica_groups=[list(range(NCORES))],
                ins=[ag1_in[:].opt()], outs=[ag1_out[:].opt()])

            # stream score tables during the all-gather
            nc.sync.dma_start(ssrcb[:], ssrcb_d)
            nc.sync.dma_start(sdstT[:], sdst_d)
            nc.sync.dma_start(E1sb[:], E1sb_d)
            nc.sync.dma_start(E2sb[:], E2sb_d)
            nc.sync.dma_start(qdT[:], qd_d)

            # h1all loads depend on AG1; mask/W2 loads are issued from the
            # gpsimd stream, which blocks on the collective completion, so
            # the big mask traffic stays off HBM until AG1 is done on every
            # core (it otherwise starves the slowest core's critical DMAs).
            ag1v = ag1_out[:].rearrange("(jc p) c -> p jc c", p=128)
            nc.sync.dma_start(h1all[:, 0:16, :], ag1v[:, 0:16, :])
            nc.sync.dma_start(h1all[:, 16:NB, :], ag1v[:, 16:NB, :])
            for jc in range(NB):
                nc.gpsimd.dma_start(maskr[:, jc, :],
                                    mT_d[jc * 128:(jc + 1) * 128, :])
            for kc in range(2):
                nc.gpsimd.dma_start(W2sb[:, kc, :],
                                    W2e_d[kc * 128:(kc + 1) * 128, :])
                nc.gpsimd.dma_start(v2sb[:, kc, :],
                                    v2s_d[kc * 128:(kc + 1) * 128, :])

            # ---------------- layer 1: two head-phases --------------------
            with tc.tile_pool(name="l1ps", bufs=1, space="PSUM") as l1ps:
                o1A = l1ps.tile([AUGH, 2, SH], f32, tag="o1A")
                o1B = l1ps.tile([AUGH, 2, SH], f32, tag="o1B")

                def emit_epilogue(ph, o1):
                    with tc.tile_pool(name=f"fin{ph}", bufs=1) as fin:
                        lnv = fin.tile([1, 2, SH], f32, tag="lnv")
                        nc.scalar.activation(lnv[:], o1[D1:D1 + 1, :, :],
                                             Af.Ln, scale=KREC)
                        rr = fin.tile([1, 2, SH], f32, tag="rr")
                        nc.scalar.activation(rr[:], lnv[:], Af.Exp, scale=-5.0)
                        zrow = fin.tile([128, SH], f32, tag="zrow")
                        for u in range(2):
                            recb = fin.tile([D1, SH], f32, tag=f"recb{u}")
                            nc.gpsimd.partition_broadcast(recb[:], rr[:, u, :])
                            nc.vector.scalar_tensor_tensor(
                                zrow[u * D1:(u + 1) * D1, :], o1[0:D1, u, :],
                                KREC, recb[:], op0=Alu.mult, op1=Alu.mult)
                        # ELU: max(z,0) + patchedExp(5*min(z,0)) - 1
                        rmax = fin.tile([128, SH], f32, tag="rmax")
                        rmin = fin.tile([128, SH], f32, tag="rmin")
                        ex = fin.tile([128, SH], f32, tag="ex")
                        nc.vector.tensor_scalar_max(rmax[:], zrow[:], 0.0)
                        nc.vector.tensor_scalar_min(rmin[:], zrow[:], 0.0)
                        nc.scalar.activation(ex[:], rmin[:], Af.Exp, scale=5.0)
                        nc.vector.scalar_tensor_tensor(
                            z1Tl[:, ph, :], ex[:], -1.0, rmax[:],
                            op0=Alu.add, op1=Alu.add)

                with tc.tile_pool(name="work", bufs=6) as wp:
                    for ph, o1 in ((0, o1A), (1, o1B)):
                        hs = [2 * ph, 2 * ph + 1]
                        for t in range(NB):
                            if ph == 1 and t == 7:
                                emit_epilogue(0, o1A)
                            jc = t
                            mb = maskr[:, jc, :].unsqueeze(1).to_broadcast(
                                (128, 2, SH))
                            # Park the first 16 phase-A chunks' matmul
                            # movings in the (still unused) pexf tile: the
                            # elementwise work is gather-independent, so the
                            # DVE can run 16 chunks ahead during the AG1
                            # rendezvous instead of stalling on pool slots
                            # released only by AG1-gated matmuls.
                            if ph == 0 and t < 16:
                                mov = pexf[:, 2 * t:2 * t + 2, :]
                            else:
                                movt = wp.tile([128, 2, SH], bf16, tag="e2")
                                mov = movt[:]
                            if t % 4 != 3:
                                pex = wp.tile([128, 2, SH], bf16, tag="e0")
                                for u, h in enumerate(hs):
                                    nc.scalar.activation(
                                        pex[:, u, :], ssrcb[:, h, :], Af.Exp,
                                        bias=sdstT[:, jc, h:h + 1])
                                nc.vector.tensor_mul(mov, pex[:], mb)
                            else:
                                t0 = wp.tile([128, 2, SH], bf16, tag="e0")
                                for u, h in enumerate(hs):
                                    nc.vector.tensor_scalar_mul(
                                        t0[:, u, :], E2sb[:, h, :],
                                        qdT[:, jc, h:h + 1])
                                t1 = wp.tile([128, 2, SH], bf16, tag="e1")
                                for u, h in enumerate(hs):
                                    nc.vector.tensor_max(
                                        t1[:, u, :], t0[:, u, :], E1sb[:, h, :])
                                nc.vector.tensor_mul(mov, t1[:], mb)
                            for u, h in enumerate(hs):
                                nc.tensor.matmul(
                                    o1[:, u, :],
                                    h1all[:, jc, AUGH * h:AUGH * (h + 1)],
                                    mov[:, u, :],
                                    start=(t == 0), stop=(t == NB - 1))

                emit_epilogue(1, o1B)

            # ---------------- layer 2: local h2 projection + all-gather ---
            with tc.tile_pool(name="s2ps", bufs=2, space="PSUM") as s2ps:
                s2p = s2ps.tile([1, SH], f32, tag="s2p", bufs=1)
                for kc in range(2):
                    nc.tensor.matmul(s2p[:], v2sb[:, kc, :], z1Tl[:, kc, :],
                                     start=(kc == 0), stop=(kc == 1))
                nc.vector.tensor_copy(s2srow[:], s2p[:])
                nc.gpsimd.partition_broadcast(s2srcb[:], s2srow[:])

                # s2_dst for the local shard, in node-partition layout
                # ([p, k] columns); tiny all-gather so the layer-2
                # exponentials can precompute during the h2 gather.
                s2dp = s2ps.tile([128, OWN], f32, tag="s2dp", bufs=1)
                for k in range(OWN):
                    for kc in range(2):
                        nc.tensor.matmul(
                            s2dp[:, k:k + 1],
                            z1Tl[:, kc, k * 128:(k + 1) * 128],
                            W2sb[:, kc, C:C + 1],
                            start=(kc == 0), stop=(kc == 1))
                nc.vector.tensor_copy(s2dcol[:], s2dp[:])
                nc.sync.dma_start(ag4_in[:], s2dcol[:])
                nc.gpsimd.collective_compute(
                    "AllGather", Alu.bypass,
                    replica_groups=[list(range(NCORES))],
                    ins=[ag4_in[:].opt()], outs=[ag4_out[:].opt()])
                ag4v = ag4_out[:].rearrange("(r p) k -> p r k", p=128)
                s2dv = s2dst[:].rearrange("p (r k) x -> p r (k x)", k=OWN)
                nc.sync.dma_start(s2dv, ag4v)
                for jc in range(NB):
                    nc.scalar.activation(pexf[:, jc, :], s2srcb[:], Af.Exp,
                                         bias=s2dst[:, jc, :])

                nc.vector.tensor_copy(
                    h2l[:, :, C:C + 1],
                    ones_col[:].unsqueeze(1).to_broadcast((128, OWN, 1)))
                for k in range(OWN):
                    h2p = s2ps.tile([128, C + 1], f32, tag="h2p")
                    for kc in range(2):
                        nc.tensor.matmul(
                            h2p[:], z1Tl[:, kc, k * 128:(k + 1) * 128],
                            W2sb[:, kc, :], start=(kc == 0), stop=(kc == 1))
                    nc.vector.tensor_copy(h2l[:, k, 0:C], h2p[:, 0:C])
                    nc.sync.dma_start(ag3_in[k * 128:(k + 1) * 128, :],
                                      h2l[:, k, :])

            nc.gpsimd.collective_compute(
                "AllGather", Alu.bypass,
                replica_groups=[list(range(NCORES))],
                ins=[ag3_in[:].opt()], outs=[ag3_out[:].opt()])
            ag3v = ag3_out[:].rearrange("(jc p) c -> p jc c", p=128)
            nc.sync.dma_start(h2all[:], ag3v)

            # ---------------- layer 2: masked softmax + aggregation -------
            with tc.tile_pool(name="aggps2", bufs=1, space="PSUM") as aggps2:
                o2 = aggps2.tile([AUGH, SH], f32)
                with tc.tile_pool(name="work2", bufs=16) as wp2:
                    for jc in range(NB):
                        pt = wp2.tile([128, SH], bf16, tag="ptb")
                        nc.vector.tensor_mul(pt[:], pexf[:, jc, :],
                                             maskr[:, jc, :])
                        nc.tensor.matmul(o2[:], h2all[:, jc, :], pt[:],
                                         start=(jc == 0), stop=(jc == NB - 1))

                with tc.tile_pool(name="fin2", bufs=1) as fin2:
                    u2 = fin2.tile([1, SH], f32, tag="u2")
                    nc.scalar.activation(u2[:], o2[D1:D1 + 1, :], Af.Ln,
                                         scale=KREC)
                    nc.scalar.activation(rec2row[:], u2[:], Af.Exp, scale=-5.0)
                    recb2 = fin2.tile([C, SH], f32, tag="recb2")
                    nc.gpsimd.partition_broadcast(recb2[:], rec2row[:])
                    outsb = fin2.tile([C, SH], f32, tag="outsb")
                    nc.vector.scalar_tensor_tensor(
                        outsb[:], o2[0:D1, :], KREC, recb2[:],
                        op0=Alu.mult, op1=Alu.mult)
                    nc.sync.dma_start(outT_d, outsb[:])

    nc.compile()
    return nc


def _get_nc():
    if "nc" not in _CACHED:
        _CACHED["nc"] = _build_nc()
    return _CACHED["nc"]


def _prep_in_maps(x, A, W1, a1_src, a1_dst, W2, a2_src, a2_dst):
    import ml_dtypes
    f = np.float32
    bf = ml_dtypes.bfloat16
    xT = np.ascontiguousarray(x.T).astype(f, copy=False)
    W1r = W1.reshape(FIN, H, D1)
    V1s = np.einsum("fhd,hd->fh", W1r, a1_src).astype(f)
    V1d = np.einsum("fhd,hd->fh", W1r, a1_dst).astype(f)
    s_src = (x @ V1s).astype(f)                    # [N, H]
    s_dst = (x @ V1d).astype(f)                    # [N, H]
    E1s = np.exp(s_src)
    E2s = np.exp(NEG * s_src)
    qd_full = np.exp(-(1.0 - NEG) * s_dst)
    E1d_full = np.exp(s_dst)

    def jlay(a):                                   # [N, H] -> [128, NB, H]
        return np.ascontiguousarray(
            a.reshape(NB, 128, H).transpose(1, 0, 2)).astype(f)

    sdstT = jlay(s_dst)
    qdT = jlay(qd_full)
    W2e = np.concatenate([W2, W2 @ a2_dst.T], axis=1).astype(bf)
    v2s = (W2 @ a2_src.T).astype(bf)

    in_maps = []
    for c in range(NCORES):
        sl = slice(c * SH, (c + 1) * SH)

        def ibc(a, dt):                  # [SH, H] rows -> [128, H, SH] bcast
            r = np.ascontiguousarray(a[sl].T)      # [H, SH]
            return np.ascontiguousarray(
                np.broadcast_to(r[None], (128, H, SH))).astype(dt)

        # E1d for the k=3 own block only (nodes 512c+384 .. 512c+512)
        E1dloc = np.ascontiguousarray(
            E1d_full[c * SH + 384:c * SH + 512][None].transpose(
                1, 0, 2)).astype(f)
        in_maps.append({
            "xsT": np.ascontiguousarray(xT[:, sl]),
            "maskT": np.ascontiguousarray((A[sl, :] > 0).T).astype(bf),
            "W1a": W1.astype(f, copy=False),
            "ssrcb": ibc(s_src, bf),
            "E1sb": ibc(E1s, bf),
            "E2sb": ibc(E2s, bf),
            "sdstT": sdstT,
            "qdT": qdT,
            "E1dloc": E1dloc,
            "W2e": W2e,
            "v2s": v2s,
        })
    return in_maps


def kernel(x, A, W1, a1_src, a1_dst, W2, a2_src, a2_dst, _want_results=False):
    from concourse.bass_utils import run_bass_kernel_spmd

    nc = _get_nc()
    in_maps = _prep_in_maps(np.asarray(x), np.asarray(A), np.asarray(W1),
                            np.asarray(a1_src), np.asarray(a1_dst),
                            np.asarray(W2), np.asarray(a2_src),
                            np.asarray(a2_dst))
    trace = bool(int(os.environ.get("GAT_TRACE", "0")))
    res = run_bass_kernel_spmd(nc, in_maps, core_ids=list(range(NCORES)),
                               trace=trace)
    out = np.empty((N, C), np.float32)
    for c in range(NCORES):
        out[c * SH:(c + 1) * SH, :] = res.results[c]["outT"].T
    if _want_results:
        return out, res
    return out


# revision 16
# speedup vs baseline: 1.2811x; 1.2811x over previous
"""GAT (2-layer graph attention network) Bass kernel for 8 trn2 NeuronCores.

Sharding: core c owns node rows [512c, 512c+512). Each core projects only its
own 512 nodes (h = x_own @ W1) and all-gathers the augmented per-head blocks;
attention exponentials are computed per j-chunk in transposed layout
[j(partitions), i(free)].

Per-chunk paths (chosen by jc % 4, so the all-gather payload is one block per
chunk) share one PSUM accumulator per head:
  jc%4<3 (ACT path): P = patchedExp(s_src[i] + s_dst[j]) * mask
  jc%4=3 (MAX path): P = max(E2s[i]*qd[j], E1s[i])*E1d[j] * mask
               via exp(lrelu(t)) = max(exp(t), exp(0.2 t)), with exp(s_dst)
               folded into the gathered stationary block (htil).
Layer 2 projects h2 for the local node shard only (from local z1 columns) and
all-gathers the small augmented [h2 | 1 | s2_dst] blocks. The patched ACT exp
table computes exp(lrelu(x)); tables needing a true exp (E1s/E2s/qd/E1d) are
host-side folds of the rank-1 score projections x @ (W1 a1_*). Softmax
reciprocals run as ln/exp on the scalar engine (rec = patchedExp(-5*ln(K*den))
= 1/(K*den)); Exp is pinned to the natural_log_exp_and_others table set so
Exp/Ln share one table load. Both layer-1 phases are emitted before their
epilogues so no in-order engine stream stalls on a PSUM-gated epilogue op
between phases.
"""

import os

import numpy as np

N, FIN, HID, H, D1, C = 4096, 512, 256, 4, 64, 64
NCORES = 8
SH = N // NCORES          # 512 local nodes per core
NB = N // 128             # 32 j-chunks
FC = FIN // 128           # 4 fin chunks
OWN = 4                   # own j-blocks per core
NEG = 0.2
AUGH = D1 + 1             # 65 per head
AUG = AUGH * H            # 260
KREC = 32.0               # reciprocal pre-scale (keeps ln(K*den) in (0, 17))

_CACHED = {}


def _make_act_root(alpha=NEG):
    """Patch the neuron ACT tables so Exp computes g(x)=exp(lrelu(x)).

    Bucket entries are [d0,d1,d2,d3,x0,0,0,0] fp32 cubics evaluated as
    y = d0+(x-x0)(d1+(x-x0)(d2+(x-x0)d3)). For exp buckets centered at
    x0<0 we substitute the Taylor cubic of exp(alpha*x) at the same
    center. Ln buckets are untouched.
    """
    import json
    import shutil
    import tempfile

    from neuronxcc.driver.Job import Job
    from neuronxcc.driver.jobs.support.FindActInfo import findActInfoFile

    src_dir = os.path.dirname(findActInfoFile(Job.getPackageDir(), "gen3"))
    dst = tempfile.mkdtemp(prefix="gat_act_root_")
    for f in os.listdir(src_dir):
        shutil.copy(os.path.join(src_dir, f), os.path.join(dst, f))
        os.chmod(os.path.join(dst, f), 0o644)
    for set_name in ("exp_and_others", "natural_log_exp_and_others",
                     "exp_and_friends"):
        meta = json.load(open(os.path.join(dst, f"{set_name}.json")))
        start = meta["func_to_bkt_start_idx"].get("exp")
        if start is None:
            continue
        nxt = [s for s in sorted(meta["func_to_bkt_start_idx"].values())
               if s > start]
        end = nxt[0] if nxt else meta["bkt_entry_cnt"]
        path = os.path.join(dst, f"{set_name}_bkt.bin")
        b = np.fromfile(path, dtype=np.float32).reshape(-1, 8).copy()
        for i in range(start, end):
            x0, d0 = float(b[i, 4]), float(b[i, 0])
            if x0 >= 0 or not np.isfinite(d0) or d0 <= 0:
                continue
            e = np.exp(alpha * x0)
            b[i, 0:4] = [e, alpha * e, alpha * alpha * e / 2.0,
                         alpha ** 3 * e / 6.0]
        b.tofile(path)
    return os.path.join(dst, "act_info.json")


def _pin_exp_table(act_root, mybir, bacc):
    """Make bacc's table-load pass see Exp only in the set that also holds
    Ln, so the whole kernel uses one ACT table load (no Exp<->Ln flips)."""
    import json

    with open(act_root) as f:
        info = json.load(f)
    tables = {}
    for ent in info["act_func_sets"]:
        fns = set()
        for v in ent["act"].keys():
            try:
                fns.add(mybir.ActivationFunctionType.from_pwp(v))
            except Exception:
                pass
        if ent["name"] != "natural_log_exp_and_others":
            fns.discard(mybir.ActivationFunctionType.Exp)
        tables[ent["name"]] = fns
    bacc.get_activation_tables = lambda arch: tables


def _build_nc():
    act_root = _make_act_root()
    os.environ["BASS_ACT_ROOT_JSON_PATH"] = act_root
    import concourse.mybir as mybir
    import concourse.tile as tile
    from concourse import bacc

    _pin_exp_table(act_root, mybir, bacc)

    f32 = mybir.dt.float32
    f32r = mybir.dt.float32r
    bf16 = mybir.dt.bfloat16
    Af = mybir.ActivationFunctionType
    Alu = mybir.AluOpType

    nc = bacc.Bacc("TRN2", target_bir_lowering=False, debug=False,
                   num_devices=NCORES)

    xsT_d = nc.dram_tensor("xsT", [FIN, SH], f32r, kind="ExternalInput").ap()
    mT_d = nc.dram_tensor("maskT", [N, SH], bf16, kind="ExternalInput").ap()
    W1_d = nc.dram_tensor("W1a", [FIN, HID], f32r, kind="ExternalInput").ap()
    ssrcb_d = nc.dram_tensor("ssrcb", [128, H, SH], bf16, kind="ExternalInput").ap()
    E1sb_d = nc.dram_tensor("E1sb", [128, H, SH], bf16, kind="ExternalInput").ap()
    E2sb_d = nc.dram_tensor("E2sb", [128, H, SH], bf16, kind="ExternalInput").ap()
    sdst_d = nc.dram_tensor("sdstT", [128, NB, H], f32, kind="ExternalInput").ap()
    qd_d = nc.dram_tensor("qdT", [128, NB, H], f32, kind="ExternalInput").ap()
    E1dl_d = nc.dram_tensor("E1dloc", [128, 1, H], f32, kind="ExternalInput").ap()
    W2e_d = nc.dram_tensor("W2e", [HID, C + 1], bf16, kind="ExternalInput").ap()
    v2s_d = nc.dram_tensor("v2s", [HID, 1], bf16, kind="ExternalInput").ap()
    outT_d = nc.dram_tensor("outT", [C, SH], f32, kind="ExternalOutput").ap()

    with tile.TileContext(nc) as tc:
        with (tc.tile_pool(name="persist", bufs=1) as pp,
              tc.tile_pool(name="dram", bufs=1, space="DRAM") as dpool):
            # ---------------- persistent SBUF tiles -----------------------
            maskr = pp.tile([128, NB, SH], bf16)
            h1all = pp.tile([128, NB, AUG], bf16)
            ssrcb = pp.tile([128, H, SH], bf16)
            E1sb = pp.tile([128, H, SH], bf16)
            E2sb = pp.tile([128, H, SH], bf16)
            sdstT = pp.tile([128, NB, H], f32)
            qdT = pp.tile([128, NB, H], f32)
            E1dloc = pp.tile([128, 1, H], f32)
            xsTt = pp.tile([128, FC, SH], f32r)
            W1sb = pp.tile([128, FC, HID], f32r)
            h1loc = pp.tile([128, OWN, AUG], bf16)
            htloc = pp.tile([128, 1, AUG], bf16)
            z1Tl = pp.tile([128, 2, SH], bf16)
            h2l = pp.tile([128, OWN, C + 1], bf16)      # [h2 | 1]
            h2all = pp.tile([128, NB, C + 1], bf16)
            s2dst = pp.tile([128, NB, 1], f32)
            s2dcol = pp.tile([128, OWN], f32)
            pexf = pp.tile([128, NB, SH], bf16)
            s2srow = pp.tile([1, SH], f32)
            s2srcb = pp.tile([128, SH], f32)
            W2sb = pp.tile([128, 2, C + 1], bf16)
            v2sb = pp.tile([128, 2, 1], bf16)
            ones_col = pp.tile([128, 1], bf16)
            rec2row = pp.tile([1, SH], f32)

            ag1_in = dpool.tile([OWN * 128, AUG], bf16)
            ag1_out = dpool.tile([N, AUG], bf16, addr_space="Shared")
            ag3_in = dpool.tile([OWN * 128, C + 1], bf16)
            ag3_out = dpool.tile([N, C + 1], bf16, addr_space="Shared")
            ag4_in = dpool.tile([128, OWN], f32)
            ag4_out = dpool.tile([128 * NCORES, OWN], f32, addr_space="Shared")

            # ---------------- AG1-critical input DMAs ---------------------
            for fc in range(FC):
                nc.sync.dma_start(xsTt[:, fc, :], xsT_d[fc * 128:(fc + 1) * 128, :])
                nc.sync.dma_start(W1sb[:, fc, :], W1_d[fc * 128:(fc + 1) * 128, :])
            nc.sync.dma_start(E1dloc[:], E1dl_d)
            nc.vector.memset(ones_col[:], 1.0)

            # ---------------- local prep: h1aug (+ htil for k=3) ----------
            with tc.tile_pool(name="ppsum", bufs=2, space="PSUM") as ppsum:
                h1v = h1loc[:].rearrange("p k (h x) -> p k h x", x=AUGH)
                nc.vector.tensor_copy(
                    h1v[:, :, :, D1:D1 + 1],
                    ones_col[:].unsqueeze(1).unsqueeze(1).to_broadcast(
                        (128, OWN, H, 1)))
                for k in range(OWN):
                    hp = ppsum.tile([128, HID], f32, tag="hp")
                    for fc in range(FC):
                        nc.tensor.matmul(
                            hp[:], xsTt[:, fc, k * 128:(k + 1) * 128],
                            W1sb[:, fc, :],
                            start=(fc == 0), stop=(fc == FC - 1))
                    nc.vector.tensor_copy(
                        h1v[:, k, :, 0:D1],
                        hp[:].rearrange("p (h d) -> p h d", h=H))
                    if k == OWN - 1:
                        for h in range(H):
                            nc.vector.tensor_scalar_mul(
                                htloc[:, 0, h * AUGH:(h + 1) * AUGH],
                                h1loc[:, k, h * AUGH:(h + 1) * AUGH],
                                E1dloc[:, 0, h:h + 1])
                        nc.sync.dma_start(ag1_in[k * 128:(k + 1) * 128, :],
                                          htloc[:, 0, :])
                    else:
                        nc.sync.dma_start(ag1_in[k * 128:(k + 1) * 128, :],
                                          h1loc[:, k, :])

            nc.gpsimd.collective_compute(
                "AllGather", Alu.bypass,
                replica_groups=[list(range(NCORES))],
                ins=[ag1_in[:].opt()], outs=[ag1_out[:].opt()])

            # stream score tables during the all-gather
            nc.sync.dma_start(ssrcb[:], ssrcb_d)
            nc.sync.dma_start(sdstT[:], sdst_d)
            nc.sync.dma_start(E1sb[:], E1sb_d)
            nc.sync.dma_start(E2sb[:], E2sb_d)
            nc.sync.dma_start(qdT[:], qd_d)

            # h1all loads depend on AG1; mask/W2 loads are issued from the
            # gpsimd stream, which blocks on the collective completion, so
            # the big mask traffic stays off HBM until AG1 is done on every
            # core (it otherwise starves the slowest core's critical DMAs).
            ag1v = ag1_out[:].rearrange("(jc p) c -> p jc c", p=128)
            for q in range(4):
                nc.sync.dma_start(h1all[:, 8 * q:8 * (q + 1), :],
                                  ag1v[:, 8 * q:8 * (q + 1), :])
            for jc in range(NB):
                nc.gpsimd.dma_start(maskr[:, jc, :],
                                    mT_d[jc * 128:(jc + 1) * 128, :])
            for kc in range(2):
                nc.gpsimd.dma_start(W2sb[:, kc, :],
                                    W2e_d[kc * 128:(kc + 1) * 128, :])
                nc.gpsimd.dma_start(v2sb[:, kc, :],
                                    v2s_d[kc * 128:(kc + 1) * 128, :])

            # ---------------- layer 1: two head-phases --------------------
            with tc.tile_pool(name="l1ps", bufs=1, space="PSUM") as l1ps:
                o1A = l1ps.tile([AUGH, 2, SH], f32, tag="o1A")
                o1B = l1ps.tile([AUGH, 2, SH], f32, tag="o1B")

                def emit_epilogue(ph, o1):
                    with tc.tile_pool(name=f"fin{ph}", bufs=1) as fin:
                        lnv = fin.tile([1, 2, SH], f32, tag="lnv")
                        nc.scalar.activation(lnv[:], o1[D1:D1 + 1, :, :],
                                             Af.Ln, scale=KREC)
                        rr = fin.tile([1, 2, SH], f32, tag="rr")
                        nc.scalar.activation(rr[:], lnv[:], Af.Exp, scale=-5.0)
                        zrow = fin.tile([128, SH], f32, tag="zrow")
                        for u in range(2):
                            recb = fin.tile([D1, SH], f32, tag=f"recb{u}")
                            nc.gpsimd.partition_broadcast(recb[:], rr[:, u, :])
                            nc.vector.scalar_tensor_tensor(
                                zrow[u * D1:(u + 1) * D1, :], o1[0:D1, u, :],
                                KREC, recb[:], op0=Alu.mult, op1=Alu.mult)
                        # ELU: max(z,0) + patchedExp(5*min(z,0)) - 1
                        rmax = fin.tile([128, SH], f32, tag="rmax")
                        rmin = fin.tile([128, SH], f32, tag="rmin")
                        ex = fin.tile([128, SH], f32, tag="ex")
                        nc.vector.tensor_scalar_max(rmax[:], zrow[:], 0.0)
                        nc.vector.tensor_scalar_min(rmin[:], zrow[:], 0.0)
                        nc.scalar.activation(ex[:], rmin[:], Af.Exp, scale=5.0)
                        nc.vector.scalar_tensor_tensor(
                            z1Tl[:, ph, :], ex[:], -1.0, rmax[:],
                            op0=Alu.add, op1=Alu.add)

                with tc.tile_pool(name="work", bufs=8) as wp:
                    for ph, o1 in ((0, o1A), (1, o1B)):
                        hs = [2 * ph, 2 * ph + 1]
                        for t in range(NB):
                            if ph == 1 and t == 7:
                                emit_epilogue(0, o1A)
                            jc = t
                            mb = maskr[:, jc, :].unsqueeze(1).to_broadcast(
                                (128, 2, SH))
                            if t % 4 != 3:
                                pex = wp.tile([128, 2, SH], bf16, tag="e0")
                                for u, h in enumerate(hs):
                                    nc.scalar.activation(
                                        pex[:, u, :], ssrcb[:, h, :], Af.Exp,
                                        bias=sdstT[:, jc, h:h + 1])
                                pt = wp.tile([128, 2, SH], bf16, tag="e2")
                                nc.vector.tensor_mul(pt[:], pex[:], mb)
                                src = pt
                            else:
                                t0 = wp.tile([128, 2, SH], bf16, tag="e0")
                                for u, h in enumerate(hs):
                                    nc.vector.tensor_scalar_mul(
                                        t0[:, u, :], E2sb[:, h, :],
                                        qdT[:, jc, h:h + 1])
                                t1 = wp.tile([128, 2, SH], bf16, tag="e1")
                                for u, h in enumerate(hs):
                                    nc.vector.tensor_max(
                                        t1[:, u, :], t0[:, u, :], E1sb[:, h, :])
                                m1 = wp.tile([128, 2, SH], bf16, tag="e2")
                                nc.vector.tensor_mul(m1[:], t1[:], mb)
                                src = m1
                            for u, h in enumerate(hs):
                                nc.tensor.matmul(
                                    o1[:, u, :],
                                    h1all[:, jc, AUGH * h:AUGH * (h + 1)],
                                    src[:, u, :],
                                    start=(t == 0), stop=(t == NB - 1))

                emit_epilogue(1, o1B)

            # ---------------- layer 2: local h2 projection + all-gather ---
            with tc.tile_pool(name="s2ps", bufs=2, space="PSUM") as s2ps:
                s2p = s2ps.tile([1, SH], f32, tag="s2p", bufs=1)
                for kc in range(2):
                    nc.tensor.matmul(s2p[:], v2sb[:, kc, :], z1Tl[:, kc, :],
                                     start=(kc == 0), stop=(kc == 1))
                nc.vector.tensor_copy(s2srow[:], s2p[:])
                nc.gpsimd.partition_broadcast(s2srcb[:], s2srow[:])

                # s2_dst for the local shard, in node-partition layout
                # ([p, k] columns); tiny all-gather so the layer-2
                # exponentials can precompute during the h2 gather.
                s2dp = s2ps.tile([128, OWN], f32, tag="s2dp", bufs=1)
                for k in range(OWN):
                    for kc in range(2):
                        nc.tensor.matmul(
                            s2dp[:, k:k + 1],
                            z1Tl[:, kc, k * 128:(k + 1) * 128],
                            W2sb[:, kc, C:C + 1],
                            start=(kc == 0), stop=(kc == 1))
                nc.vector.tensor_copy(s2dcol[:], s2dp[:])
                nc.sync.dma_start(ag4_in[:], s2dcol[:])
                nc.gpsimd.collective_compute(
                    "AllGather", Alu.bypass,
                    replica_groups=[list(range(NCORES))],
                    ins=[ag4_in[:].opt()], outs=[ag4_out[:].opt()])
                ag4v = ag4_out[:].rearrange("(r p) k -> p r k", p=128)
                s2dv = s2dst[:].rearrange("p (r k) x -> p r (k x)", k=OWN)
                nc.sync.dma_start(s2dv, ag4v)
                for jc in range(NB):
                    nc.scalar.activation(pexf[:, jc, :], s2srcb[:], Af.Exp,
                                         bias=s2dst[:, jc, :])

                nc.vector.tensor_copy(
                    h2l[:, :, C:C + 1],
                    ones_col[:].unsqueeze(1).to_broadcast((128, OWN, 1)))
                for k in range(OWN):
                    h2p = s2ps.tile([128, C + 1], f32, tag="h2p")
                    for kc in range(2):
                        nc.tensor.matmul(
                            h2p[:], z1Tl[:, kc, k * 128:(k + 1) * 128],
                            W2sb[:, kc, :], start=(kc == 0), stop=(kc == 1))
                    nc.vector.tensor_copy(h2l[:, k, 0:C], h2p[:, 0:C])
                    nc.sync.dma_start(ag3_in[k * 128:(k + 1) * 128, :],
                                      h2l[:, k, :])

            nc.gpsimd.collective_compute(
                "AllGather", Alu.bypass,
                replica_groups=[list(range(NCORES))],
                ins=[ag3_in[:].opt()], outs=[ag3_out[:].opt()])
            ag3v = ag3_out[:].rearrange("(jc p) c -> p jc c", p=128)
            nc.sync.dma_start(h2all[:], ag3v)

            # ---------------- layer 2: masked softmax + aggregation -------
            with tc.tile_pool(name="aggps2", bufs=1, space="PSUM") as aggps2:
                o2 = aggps2.tile([AUGH, SH], f32)
                with tc.tile_pool(name="work2", bufs=16) as wp2:
                    for jc in range(NB):
                        pt = wp2.tile([128, SH], bf16, tag="ptb")
                        nc.vector.tensor_mul(pt[:], pexf[:, jc, :],
                                             maskr[:, jc, :])
                        nc.tensor.matmul(o2[:], h2all[:, jc, :], pt[:],
                                         start=(jc == 0), stop=(jc == NB - 1))

                with tc.tile_pool(name="fin2", bufs=1) as fin2:
                    u2 = fin2.tile([1, SH], f32, tag="u2")
                    nc.scalar.activation(u2[:], o2[D1:D1 + 1, :], Af.Ln,
                                         scale=KREC)
                    nc.scalar.activation(rec2row[:], u2[:], Af.Exp, scale=-5.0)
                    recb2 = fin2.tile([C, SH], f32, tag="recb2")
                    nc.gpsimd.partition_broadcast(recb2[:], rec2row[:])
                    outsb = fin2.tile([C, SH], f32, tag="outsb")
                    nc.vector.scalar_tensor_tensor(
                        outsb[:], o2[0:D1, :], KREC, recb2[:],
                        op0=Alu.mult, op1=Alu.mult)
                    nc.sync.dma_start(outT_d, outsb[:])

    nc.compile()
    return nc


def _get_nc():
    if "nc" not in _CACHED:
        _CACHED["nc"] = _build_nc()
    return _CACHED["nc"]


def _prep_in_maps(x, A, W1, a1_src, a1_dst, W2, a2_src, a2_dst):
    import ml_dtypes
    f = np.float32
    bf = ml_dtypes.bfloat16
    xT = np.ascontiguousarray(x.T).astype(f, copy=False)
    W1r = W1.reshape(FIN, H, D1)
    V1s = np.einsum("fhd,hd->fh", W1r, a1_src).astype(f)
    V1d = np.einsum("fhd,hd->fh", W1r, a1_dst).astype(f)
    s_src = (x @ V1s).astype(f)                    # [N, H]
    s_dst = (x @ V1d).astype(f)                    # [N, H]
    E1s = np.exp(s_src)
    E2s = np.exp(NEG * s_src)
    qd_full = np.exp(-(1.0 - NEG) * s_dst)
    E1d_full = np.exp(s_dst)

    def jlay(a):                                   # [N, H] -> [128, NB, H]
        return np.ascontiguousarray(
            a.reshape(NB, 128, H).transpose(1, 0, 2)).astype(f)

    sdstT = jlay(s_dst)
    qdT = jlay(qd_full)
    W2e = np.concatenate([W2, W2 @ a2_dst.T], axis=1).astype(bf)
    v2s = (W2 @ a2_src.T).astype(bf)

    in_maps = []
    for c in range(NCORES):
        sl = slice(c * SH, (c + 1) * SH)

        def ibc(a, dt):                  # [SH, H] rows -> [128, H, SH] bcast
            r = np.ascontiguousarray(a[sl].T)      # [H, SH]
            return np.ascontiguousarray(
                np.broadcast_to(r[None], (128, H, SH))).astype(dt)

        # E1d for the k=3 own block only (nodes 512c+384 .. 512c+512)
        E1dloc = np.ascontiguousarray(
            E1d_full[c * SH + 384:c * SH + 512][None].transpose(
                1, 0, 2)).astype(f)
        in_maps.append({
            "xsT": np.ascontiguousarray(xT[:, sl]),
            "maskT": np.ascontiguousarray((A[sl, :] > 0).T).astype(bf),
            "W1a": W1.astype(f, copy=False),
            "ssrcb": ibc(s_src, bf),
            "E1sb": ibc(E1s, bf),
            "E2sb": ibc(E2s, bf),
            "sdstT": sdstT,
            "qdT": qdT,
            "E1dloc": E1dloc,
            "W2e": W2e,
            "v2s": v2s,
        })
    return in_maps


def kernel(x, A, W1, a1_src, a1_dst, W2, a2_src, a2_dst, _want_results=False):
    from concourse.bass_utils import run_bass_kernel_spmd

    nc = _get_nc()
    in_maps = _prep_in_maps(np.asarray(x), np.asarray(A), np.asarray(W1),
                            np.asarray(a1_src), np.asarray(a1_dst),
                            np.asarray(W2), np.asarray(a2_src),
                            np.asarray(a2_dst))
    trace = bool(int(os.environ.get("GAT_TRACE", "0")))
    res = run_bass_kernel_spmd(nc, in_maps, core_ids=list(range(NCORES)),
                               trace=trace)
    out = np.empty((N, C), np.float32)
    for c in range(NCORES):
        out[c * SH:(c + 1) * SH, :] = res.results[c]["outT"].T
    if _want_results:
        return out, res
    return out
